# revision 3
# baseline (speedup 1.0000x reference)
"""InterpretableMultiHeadAttention Trainium2 kernel (8-core SPMD).

Sharding: attention rows (Sq) are sharded across the 8 cores (256 rows
per batch per core).  The K/V projections are sharded over Sk and the
projected K^T / V' tiles are exchanged with a single AllGather, so no
projection work is replicated.  Within a core, per (batch, q-tile,
head): PE computes the logits plus a mask add (identity-matmul PSUM
accumulate), ScalarE does exp with a fused row-sum (accum_out), and one
DVE scalar_tensor_tensor accumulates the normalized probabilities into
the head-mean.  The context and output projections run per q-tile on
the transposed accumulator.
"""

import sys

sys.path.insert(0, "/opt/trn_rl_repo")

import numpy as np

import concourse.bass as bass
import concourse.mybir as mybir
import concourse.tile as tile
from concourse import bacc, bass_utils

B, S, D, H = 2, 2048, 1024, 16
DEPTH = D // H  # 64
NCORES = 8
QS = S // NCORES            # 256 q-rows per batch per core
QT = QS // 128              # 2 q-tiles of 128
HP = H // 2                 # 8 head pairs
DC = D // 128               # 8 contraction chunks
NEG = -30000.0              # mask fill (after exp scale); exp -> exactly 0
KV_ROWS = B * HP * 128      # gather rows holding KHT slices
VP_ROWS = 128               # one row-block packing the vp (b, kt) slices
GIN_ROWS = KV_ROWS + VP_ROWS

f32 = mybir.dt.float32
f32r = mybir.dt.float32r

_CACHE = {}


def _build():
    nc = bacc.Bacc("TRN2", target_bir_lowering=False, debug=False,
                   num_devices=NCORES)

    # -------- I/O --------
    qT = nc.dram_tensor("qT", [B, D, QS], f32, kind="ExternalInput").ap()
    kT = nc.dram_tensor("kT", [B, D, QS], f32, kind="ExternalInput").ap()
    vT = nc.dram_tensor("vT", [B, D, QS], f32, kind="ExternalInput").ap()
    mneg = nc.dram_tensor("mneg", [QS, S], f32, kind="ExternalInput").ap()
    wq = nc.dram_tensor("wq", [D, D], f32, kind="ExternalInput").ap()
    wk = nc.dram_tensor("wk", [D, D], f32, kind="ExternalInput").ap()
    wv = nc.dram_tensor("wv", [D, DEPTH], f32, kind="ExternalInput").ap()
    wo = nc.dram_tensor("wo", [DEPTH, D], f32, kind="ExternalInput").ap()
    bq = nc.dram_tensor("bq", [128, DC], f32, kind="ExternalInput").ap()
    bk = nc.dram_tensor("bk", [128, DC], f32, kind="ExternalInput").ap()
    bv = nc.dram_tensor("bv", [DEPTH, 1], f32, kind="ExternalInput").ap()
    bo = nc.dram_tensor("bo", [1, D], f32, kind="ExternalInput").ap()
    ident_in = nc.dram_tensor("ident", [128, 128], f32,
                              kind="ExternalInput").ap()
    ones_in = nc.dram_tensor("ones", [1, 128], f32, kind="ExternalInput").ap()

    attn_o = nc.dram_tensor("attn", [B, QS, S], f32, kind="ExternalOutput").ap()
    out_o = nc.dram_tensor("o", [B, QS, D], f32, kind="ExternalOutput").ap()

    with tile.TileContext(nc) as tc:
        with tc.tile_pool(name="const", bufs=1) as cpool, \
             tc.tile_pool(name="psum", bufs=2, space="PSUM") as psp, \
             tc.tile_pool(name="dram", bufs=1, space="DRAM") as dram:

            # persistent constants
            wo_sb = cpool.tile([DEPTH, D], f32r, tag="wo")
            nc.sync.dma_start(wo_sb[:], wo.bitcast(f32r))
            ident = cpool.tile([128, 128], f32r, tag="ident")
            nc.sync.dma_start(ident[:], ident_in.bitcast(f32r))
            identf = cpool.tile([128, 128], f32, tag="identf")
            nc.sync.dma_start(identf[:], ident_in)
            ones_sb = cpool.tile([1, 128], f32r, tag="ones")
            nc.sync.dma_start(ones_sb[:], ones_in.bitcast(f32r))
            bv_sb = cpool.tile([DEPTH, 1], f32, tag="bv")
            bo_sb = cpool.tile([1, D], f32r, tag="bo")
            nc.sync.dma_start(bv_sb[:], bv)
            nc.sync.dma_start(bo_sb[:], bo.bitcast(f32r))

            # mask rows for this core's q-slice, as f32r [qt][128, S]
            mneg_sb = cpool.tile([128, QT, S], f32r, tag="mneg")
            nc.sync.dma_start(
                mneg_sb[:], mneg.rearrange("(t p) n -> p t n", p=128).bitcast(f32r))

            qht_sb = cpool.tile([128, B, HP, QS], f32r, tag="qht")
            vp_sb = cpool.tile([128, B, 16, DEPTH], f32r, tag="vp")

            gin = dram.tile([GIN_ROWS, QS], f32r)
            gout = dram.tile([NCORES * GIN_ROWS, QS], f32r)

            # -------- projections (K/V sharded over Sk) --------
            with tc.tile_pool(name="proj", bufs=1) as projpool:
                wq_sb = projpool.tile([128, DC, D], f32r, tag="wq")
                wk_sb = projpool.tile([128, DC, D], f32r, tag="wk")
                nc.sync.dma_start(
                    wq_sb[:], wq.rearrange("(c p) n -> p c n", p=128).bitcast(f32r))
                nc.sync.dma_start(
                    wk_sb[:], wk.rearrange("(c p) n -> p c n", p=128).bitcast(f32r))
                wv_sb = projpool.tile([128, DC, DEPTH], f32r, tag="wv")
                nc.sync.dma_start(
                    wv_sb[:], wv.rearrange("(c p) n -> p c n", p=128).bitcast(f32r))
                bq_sb = projpool.tile([128, DC], f32, tag="bq")
                bk_sb = projpool.tile([128, DC], f32, tag="bk")
                nc.sync.dma_start(bq_sb[:], bq)
                nc.sync.dma_start(bk_sb[:], bk)

                xT_sb = projpool.tile([128, B, 3, DC, QS], f32r, tag="xT")
                for b in range(B):
                    nc.sync.dma_start(
                        xT_sb[:, b, 0],
                        qT[b].rearrange("(c p) n -> p c n", p=128).bitcast(f32r))
                    nc.sync.dma_start(
                        xT_sb[:, b, 1],
                        kT[b].rearrange("(c p) n -> p c n", p=128).bitcast(f32r))
                    nc.sync.dma_start(
                        xT_sb[:, b, 2],
                        vT[b].rearrange("(c p) n -> p c n", p=128).bitcast(f32r))

                # V projection -> vp_pack [128, (b,kt)*DEPTH] == [128, QS]
                vp_pack = projpool.tile([128, B * 2 * DEPTH], f32r, tag="vp_pack")
                for b in range(B):
                    for kt in range(2):
                        ps_v = psp.tile([128, DEPTH], f32, tag="ps")
                        for dc in range(DC):
                            nc.tensor.matmul(
                                ps_v[:],
                                xT_sb[:, b, 2, dc, kt * 128:(kt + 1) * 128],
                                wv_sb[:, dc],
                                start=(dc == 0), stop=(dc == DC - 1))
                        nc.scalar.activation(
                            vp_pack[:, (b * 2 + kt) * DEPTH:(b * 2 + kt + 1) * DEPTH],
                            ps_v[:], mybir.ActivationFunctionType.Copy)
                nc.sync.dma_start(gin[KV_ROWS:GIN_ROWS, :], vp_pack[:])

                # K projection for this core's Sk slice -> gather input
                for b in range(B):
                    for hp in range(HP):
                        ps_k = psp.tile([128, QS], f32, tag="ps")
                        for dc in range(DC):
                            nc.tensor.matmul(
                                ps_k[:],
                                wk_sb[:, dc, hp * 128:(hp + 1) * 128],
                                xT_sb[:, b, 1, dc],
                                start=(dc == 0), stop=(dc == DC - 1))
                        kht_sl = projpool.tile([128, QS], f32r, tag="kht_sl",
                                               bufs=3)
                        nc.scalar.activation(
                            kht_sl[:], ps_k[:],
                            mybir.ActivationFunctionType.Identity,
                            bias=bk_sb[:, hp:hp + 1])
                        nc.sync.dma_start(
                            gin[(b * HP + hp) * 128:(b * HP + hp + 1) * 128, :],
                            kht_sl[:])

                # one AllGather for KHT slices + vp slices
                nc.gpsimd.collective_compute(
                    "AllGather",
                    mybir.AluOpType.bypass,
                    ins=[gin[:].opt()],
                    outs=[gout[:].opt()],
                    replica_groups=[list(range(NCORES))],
                )

                # Q projection (only this core's q rows); overlaps the gather
                for b in range(B):
                    for hp in range(HP):
                        ps_q = psp.tile([128, QS], f32, tag="ps")
                        for dc in range(DC):
                            nc.tensor.matmul(
                                ps_q[:],
                                wq_sb[:, dc, hp * 128:(hp + 1) * 128],
                                xT_sb[:, b, 0, dc],
                                start=(dc == 0), stop=(dc == DC - 1))
                        nc.scalar.activation(
                            qht_sb[:, b, hp], ps_q[:],
                            mybir.ActivationFunctionType.Identity,
                            bias=bq_sb[:, hp:hp + 1])

            # gathered views
            g_by_core = gout[:].rearrange("(c r) n -> r c n", c=NCORES)

            # -------- attention --------
            with tc.tile_pool(name="khtp", bufs=1) as khtpool, \
                 tc.tile_pool(name="work", bufs=2) as wpool, \
                 tc.tile_pool(name="ppool", bufs=3) as ppool:
                for b in range(B):
                    for c in range(NCORES):
                        for kt in range(2):
                            nc.sync.dma_start(
                                vp_sb[:, b, c * 2 + kt],
                                g_by_core[KV_ROWS:KV_ROWS + 128, c,
                                          (b * 2 + kt) * DEPTH:(b * 2 + kt + 1) * DEPTH])
                    kht_b = khtpool.tile([128, HP, NCORES, QS], f32r, tag="kht")
                    for hp in range(HP):
                        nc.sync.dma_start(
                            kht_b[:, hp],
                            g_by_core[(b * HP + hp) * 128:(b * HP + hp + 1) * 128])
                    kht_flat = kht_b[:].rearrange("p h c n -> p h (c n)")

                    for qt in range(QT):
                        acc = wpool.tile([128, S], f32, tag="acc", bufs=2)
                        for h in range(H):
                            hp, j = h // 2, h % 2
                            ps_l = psp.tile([128, S], f32, tag="ps")
                            for n5 in range(4):
                                nsl = slice(n5 * 512, (n5 + 1) * 512)
                                nc.tensor.matmul(
                                    ps_l[:, nsl],
                                    qht_sb[j * 64:(j + 1) * 64, b, hp,
                                           qt * 128:(qt + 1) * 128],
                                    kht_flat[j * 64:(j + 1) * 64, hp, nsl],
                                    start=True, stop=False)
                                nc.tensor.matmul(
                                    ps_l[:, nsl],
                                    ident[:],
                                    mneg_sb[:, qt, nsl],
                                    start=False, stop=True)
                            p_sb = ppool.tile([128, S], f32, tag="p")
                            rs = ppool.tile([128, 1], f32, tag="rs")
                            nc.scalar.activation(
                                p_sb[:], ps_l[:],
                                mybir.ActivationFunctionType.Exp,
                                scale=0.125, accum_out=rs[:])
                            rec = ppool.tile([128, 1], f32, tag="rec")
                            nc.vector.reciprocal(rec[:], rs[:])
                            rec16 = ppool.tile([128, 1], f32, tag="rec16")
                            nc.vector.tensor_scalar_mul(rec16[:], rec[:], 1.0 / H)
                            if h == 0:
                                nc.vector.tensor_scalar(
                                    acc[:], p_sb[:], rec16[:], None,
                                    op0=mybir.AluOpType.mult)
                            else:
                                nc.vector.scalar_tensor_tensor(
                                    acc[:], p_sb[:], rec16[:], acc[:],
                                    op0=mybir.AluOpType.mult,
                                    op1=mybir.AluOpType.add)

                        # attn_mean out
                        nc.sync.dma_start(
                            attn_o[b, qt * 128:(qt + 1) * 128, :], acc[:])

                        # transpose acc -> accT (f32r) via PE
                        accT = wpool.tile([128, 16, 128], f32r, tag="accT",
                                          bufs=2)
                        for n5 in range(4):
                            ps_t = psp.tile([128, 512], f32, tag="ps")
                            for i in range(4):
                                nc.tensor.transpose(
                                    ps_t[:, i * 128:(i + 1) * 128],
                                    acc[:, (n5 * 4 + i) * 128:(n5 * 4 + i + 1) * 128],
                                    identf[:])
                            for i in range(4):
                                nc.vector.tensor_copy(
                                    accT[:, n5 * 4 + i],
                                    ps_t[:, i * 128:(i + 1) * 128])

                        # ctx^T [DEPTH, 128]
                        ps_c = psp.tile([DEPTH, 128], f32, tag="ps")
                        for g in range(16):
                            nc.tensor.matmul(
                                ps_c[:], vp_sb[:, b, g], accT[:, g],
                                start=(g == 0), stop=(g == 15))
                        ctxT = wpool.tile([DEPTH, 128], f32r, tag="ctxT", bufs=2)
                        nc.scalar.activation(
                            ctxT[:], ps_c[:],
                            mybir.ActivationFunctionType.Identity,
                            bias=bv_sb[:])

                        # out = ctx @ Wo + bo
                        ps_o = psp.tile([128, D], f32, tag="ps")
                        for n5 in range(2):
                            nsl = slice(n5 * 512, (n5 + 1) * 512)
                            nc.tensor.matmul(
                                ps_o[:, nsl], ctxT[:], wo_sb[:, nsl],
                                start=True, stop=False)
                            nc.tensor.matmul(
                                ps_o[:, nsl], ones_sb[:], bo_sb[:, nsl],
                                start=False, stop=True)
                        out_sb = wpool.tile([128, D], f32, tag="out_sb", bufs=2)
                        nc.vector.tensor_copy(out_sb[:], ps_o[:])
                        nc.sync.dma_start(
                            out_o[b, qt * 128:(qt + 1) * 128, :], out_sb[:])

    nc.compile()
    return nc


def _prep_inputs(q, k, v, mask, Wq, bq, Wk, bk, Wv, bv, Wo, bo):
    qT = np.ascontiguousarray(q.transpose(0, 2, 1)).astype(np.float32)
    kT = np.ascontiguousarray(k.transpose(0, 2, 1)).astype(np.float32)
    vT = np.ascontiguousarray(v.transpose(0, 2, 1)).astype(np.float32)
    # device applies exp(0.125 * (qk + mneg)); want 0.125*mneg == mask*NEG
    mneg_h = mask.reshape(S, S).astype(np.float32) * (NEG * 8.0)
    bq_h = np.ascontiguousarray(np.asarray(bq, np.float32).reshape(DC, 128).T)
    bk_h = np.ascontiguousarray(np.asarray(bk, np.float32).reshape(DC, 128).T)
    bv_h = np.ascontiguousarray(np.asarray(bv, np.float32).reshape(DEPTH, 1))
    bo_h = np.ascontiguousarray(np.asarray(bo, np.float32).reshape(1, D))
    ident = np.eye(128, dtype=np.float32)
    ones = np.ones((1, 128), dtype=np.float32)

    in_maps = []
    for c in range(NCORES):
        sl = slice(c * QS, (c + 1) * QS)
        in_maps.append({
            "qT": np.ascontiguousarray(qT[:, :, sl]),
            "kT": np.ascontiguousarray(kT[:, :, sl]),
            "vT": np.ascontiguousarray(vT[:, :, sl]),
            "mneg": np.ascontiguousarray(mneg_h[sl, :]),
            "wq": np.asarray(Wq, np.float32), "wk": np.asarray(Wk, np.float32),
            "wv": np.asarray(Wv, np.float32), "wo": np.asarray(Wo, np.float32),
            "bq": bq_h, "bk": bk_h, "bv": bv_h, "bo": bo_h,
            "ident": ident, "ones": ones,
        })
    return in_maps


def build_module():
    if "nc" not in _CACHE:
        _CACHE["nc"] = _build()
    return _CACHE["nc"]


def kernel(q, k, v, mask, Wq, bq, Wk, bk, Wv, bv, Wo, bo):
    nc = build_module()
    in_maps = _prep_inputs(q, k, v, mask, Wq, bq, Wk, bk, Wv, bv, Wo, bo)
    res = bass_utils.run_bass_kernel_spmd(
        nc, in_maps, core_ids=list(range(NCORES)))
    out = np.concatenate([res.results[c]["o"] for c in range(NCORES)], axis=1)
    attn = np.concatenate([res.results[c]["attn"] for c in range(NCORES)],
                          axis=1)
    return out.astype(np.float32), attn.astype(np.float32)
